# revision 32
# baseline (speedup 1.0000x reference)
"""Trainium2 Bass kernel for nn_CMF: per-channel spatial row-attention + 1x1 convs.

Reference (B=16, C=768, H=W=56):
  q = Wq @ x_s ; k = Wk @ x_fq ; v = Wv @ x_fq        (1x1 convs)
  scores[b,c,h,g] = sum_w q[b,c,h,w] k[b,c,g,w] * (H*W*C)**-0.5
  attn = softmax(scores, -1); fuse = attn @ v
  out = W1 @ zero_pad(x_s + x_mt + fuse, 1) + b1      -> (B, C, 58, 58)

Sharding: data-parallel over batch; 2 images per core on 8 cores (SPMD).

Per-core pipeline (per image, channel blocks of 128 processed in halves):
  A) channel-mix matmuls (bf16) in natural layout -> q/k/v nat tiles
  B) pad-copy (GpSimd) to w128-padded staging; xbar DMA-transpose to
     QT/KT [w(part), h, c] and VH [h(part), w, c]; per-channel attention:
       scoresT = kT.T @ qT  ->  exp(scale*x) on ACT  ->
       sums broadcast to all partitions via all-ones matmul -> reciprocal ->
       fuseT = v.T-form matmul; normalize+scatter into FT [w, h, c]
     back-xbar to natural [c, h, w64]; bounce via DRAM
  C) s = x_s + x_mt + fuse; conv (bf16) + bias on ACT; border rows/cols = b1
"""

import sys

import numpy as np

sys.path.insert(0, "/opt/trn_rl_repo")

N_CORES = 8


class Cfg:
    def __init__(self, imgs=2, cb=6, h=56, w=56, rows_per_tile=8, half=2,
                 nat_bufs=6, sim_safe=False, phases="ABC"):
        self.sim_safe = sim_safe
        self.phases = phases
        self.imgs = imgs
        self.cb = cb
        self.C = cb * 128
        self.H = h
        self.W = w
        self.S = h * w
        self.RT = rows_per_tile
        assert h % rows_per_tile == 0
        self.NT = h // rows_per_tile
        self.NS = rows_per_tile * w
        assert self.NS <= 512
        self.GC = max(1, min(512 // h, 8))
        assert 128 % self.GC == 0
        self.NG = 128 // self.GC
        self.scale = float((h * w * self.C) ** -0.5)
        self.HP = h + 2
        self.WP = w + 2
        self.half = half
        self.nat_bufs = nat_bufs


def build_program(cfg):
    from contextlib import ExitStack

    import concourse.bass as bass
    import concourse.mybir as mybir
    import concourse.tile as tile

    f32 = mybir.dt.float32
    bf16 = mybir.dt.bfloat16
    AF = mybir.ActivationFunctionType
    ALU = mybir.AluOpType

    nc = bass.Bass()

    IM, CB, H, W, S = cfg.imgs, cfg.cb, cfg.H, cfg.W, cfg.S
    RT, NT, NS, C, HP = cfg.RT, cfg.NT, cfg.NS, cfg.C, cfg.HP
    WP = cfg.WP
    GC, NG = cfg.GC, cfg.NG
    WF = 64 if W <= 64 else 128  # fuse natural w stride (back-xbar minor dim)

    x_s = nc.declare_dram_parameter("x_s", [IM, C, S], f32, isOutput=False)
    x_fq = nc.declare_dram_parameter("x_fq", [IM, C, S], f32, isOutput=False)
    x_mt = nc.declare_dram_parameter("x_mt", [IM, C, S], f32, isOutput=False)
    wqT = nc.declare_dram_parameter("wqT", [C, C], f32, isOutput=False)
    wkT = nc.declare_dram_parameter("wkT", [C, C], f32, isOutput=False)
    wvT = nc.declare_dram_parameter("wvT", [C, C], f32, isOutput=False)
    w1T = nc.declare_dram_parameter("w1T", [C, C], f32, isOutput=False)
    b1 = nc.declare_dram_parameter("b1", [C], f32, isOutput=False)
    y = nc.declare_dram_parameter("y", [IM, C, HP, WP], f32, isOutput=True)

    fnat_d = nc.dram_tensor("fnat_d", [IM, CB, 128, H, W], bf16)

    with tile.TileContext(nc) as tc, ExitStack() as ex:
        wpool = ex.enter_context(tc.tile_pool(name="wpool", bufs=1))
        xpool = ex.enter_context(tc.tile_pool(name="xpool", bufs=2))
        natpool = ex.enter_context(tc.tile_pool(name="natpool", bufs=2))
        sgpool = ex.enter_context(tc.tile_pool(name="sgpool", bufs=4))
        bigpool = ex.enter_context(tc.tile_pool(name="bigpool", bufs=2))
        tpool = ex.enter_context(tc.tile_pool(name="tpool", bufs=1))
        epool = ex.enter_context(tc.tile_pool(name="epool", bufs=2))
        rpool = ex.enter_context(tc.tile_pool(name="rpool", bufs=2))
        ftpool = ex.enter_context(tc.tile_pool(name="ftpool", bufs=1))
        fnpool = ex.enter_context(tc.tile_pool(name="fnpool", bufs=1))
        mps = ex.enter_context(tc.tile_pool(name="mps", bufs=3, space="PSUM"))
        sps = ex.enter_context(tc.tile_pool(name="sps", bufs=2, space="PSUM"))
        fps = ex.enter_context(tc.tile_pool(name="fps", bufs=2, space="PSUM"))
        bps = ex.enter_context(tc.tile_pool(name="bps", bufs=1, space="PSUM"))

        # ---- resident weights / constants ----
        wq = wpool.tile([128, CB, C], bf16)
        wk = wpool.tile([128, CB, C], bf16)
        wv = wpool.tile([128, CB, C], bf16)
        w1 = wpool.tile([128, CB, C], bf16)
        for dst, src in ((wq, wqT), (wk, wkT), (wv, wvT), (w1, w1T)):
            nc.gpsimd.dma_start(
                out=dst, in_=src.rearrange("(kb p) o -> p kb o", p=128))
        b1t = wpool.tile([128, CB], f32)
        nc.gpsimd.dma_start(out=b1t, in_=b1.rearrange("(kb p) -> p kb", p=128))
        onesm = wpool.tile([128, 64], bf16)
        nc.vector.memset(onesm, 1.0)
        BW = max(2 * H, WP)
        bord = wpool.tile([128, CB, BW], bf16)
        nc.vector.tensor_copy(
            out=bord,
            in_=bass.AP(tensor=b1t.tensor, offset=b1t.offset,
                        ap=[list(b1t.ap[0]), list(b1t.ap[1]), [0, BW]]))

        for img in range(IM):
            xs_i = x_s[img].rearrange("(kb p) s -> p kb s", p=128)
            xfq_i = x_fq[img].rearrange("(kb p) s -> p kb s", p=128)
            for h0 in range(0, CB, cfg.half):
                obs = list(range(h0, min(h0 + cfg.half, CB)))
                nats = {}
                # ---- phase A: channel mix ----
                # q/k: stage one n-tile in a small rotating buffer and
                # xbar it immediately into QT/KT -- the transposes
                # pipeline behind the mix matmuls instead of serializing
                # as one big per-o transpose after phase A
                for n in range(NT):
                    xs_t = xpool.tile([128, CB, NS], bf16, tag="xs")
                    xfq_t = xpool.tile([128, CB, NS], bf16, tag="xfq", bufs=2)
                    nc.gpsimd.dma_start(
                        out=xs_t, in_=xs_i[:, :, n * NS:(n + 1) * NS])
                    nc.gpsimd.dma_start(
                        out=xfq_t, in_=xfq_i[:, :, n * NS:(n + 1) * NS])
                    for o in obs:
                        if n == 0:
                            nats[o] = (
                                tpool.tile([128, H, 64], bf16, tag="qt",
                                           bufs=2, name=f"QT_{img}_{o}"),
                                tpool.tile([128, H, 64], bf16, tag="kt",
                                           bufs=2, name=f"KT_{img}_{o}"),
                                natpool.tile([64, S], bf16, tag="nat",
                                             name=f"vnat_{img}_{o}"),
                                natpool.tile([64, S], bf16, tag="nat2",
                                             bufs=2, name=f"vn2_{img}_{o}"),
                                bigpool.tile([64, W, 128], bf16, tag="big",
                                             name=f"vstg_{img}_{o}"),
                            )
                        QT, KT, vn, vn2, vstg = nats[o]
                        for ti, (wmat, xt) in enumerate(
                                ((wq, xs_t), (wk, xfq_t), (wv, xfq_t))):
                            ps = mps.tile([128, NS], f32, tag="mixps")
                            for kb in range(CB):
                                nc.tensor.matmul(
                                    ps,
                                    lhsT=wmat[:, kb, o * 128:(o + 1) * 128],
                                    rhs=xt[:, kb, :],
                                    start=(kb == 0), stop=(kb == CB - 1))
                            if ti == 2:
                                nc.vector.tensor_copy(
                                    out=vn[:, n * NS:(n + 1) * NS],
                                    in_=ps[0:64])
                                nc.vector.tensor_copy(
                                    out=vn2[:, n * NS:(n + 1) * NS],
                                    in_=ps[64:128])
                                # scatter v rows into the h-packed
                                # transpose staging as they arrive
                                nc.gpsimd.tensor_copy(
                                    out=vstg[:, :, n * RT:(n + 1) * RT],
                                    in_=vn[:, n * NS:(n + 1) * NS
                                           ].rearrange(
                                        "p (h w) -> p w h", h=RT))
                                nc.gpsimd.tensor_copy(
                                    out=vstg[:, :,
                                             64 + n * RT:64 + (n + 1) * RT],
                                    in_=vn2[:, n * NS:(n + 1) * NS
                                            ].rearrange(
                                        "p (h w) -> p w h", h=RT))
                            else:
                                # channel-pair packing: channels 0-63 at
                                # w-cols 0:56, 64-127 at 64:64+56, so one
                                # xbar slice covers all 128 channels
                                stg = sgpool.tile([64, RT, 128], bf16,
                                                  tag="sg", bufs=4)
                                nc.vector.tensor_copy(
                                    out=stg[:, :, 0:W],
                                    in_=ps[0:64].rearrange(
                                        "p (h w) -> p h w", h=RT))
                                nc.vector.tensor_copy(
                                    out=stg[:, :, 64:64 + W],
                                    in_=ps[64:128].rearrange(
                                        "p (h w) -> p h w", h=RT))
                                nc.sync.dma_start(
                                    out=nats[o][ti][
                                        :, n * RT:(n + 1) * RT, :],
                                    in_=stg.rearrange("p a b -> p (a b)"),
                                    transpose=True)
                # ---- phase B: attention ----
                # channel halves A (0-63) / B (64-127) sit at partition
                # offsets 0 / 64 of QT/KT/VHh; A and B matmuls alternate
                # row+col groups so LDWEIGHTS overlaps in-flight matmuls
                for o in (obs if "B" in cfg.phases else []):
                    QT, KT, vn, vn2, vstg = nats[o]
                    do_xbar = "x" not in cfg.phases
                    do_attn = "a" not in cfg.phases
                    FT = ftpool.tile([64, H, 128], bf16, tag="ft", bufs=2)
                    VHh = tpool.tile([128, W, 64], bf16, tag="vh", bufs=2)
                    if do_xbar:
                        nc.sync.dma_start(
                            out=VHh,
                            in_=vstg.rearrange("p a b -> p (a b)"),
                            transpose=True)
                    for g in (range(NG // 2) if (do_xbar and do_attn)
                              else []):
                        c0 = g * GC
                        sp = sps.tile([128, GC * H], f32, tag="sps")
                        for ci in range(GC):
                            for off in (0, 64):
                                nc.tensor.matmul(
                                    sp[off:off + H,
                                       ci * H:(ci + 1) * H],
                                    lhsT=KT[off:off + W, :, c0 + ci],
                                    rhs=QT[off:off + W, :, c0 + ci],
                                    start=True, stop=True,
                                    tile_position=(off, off))
                        et = epool.tile([128, GC * H], bf16, tag="exp",
                                        bufs=2)
                        nc.scalar.activation(
                            out=et, in_=sp, func=AF.Exp, scale=cfg.scale)
                        bp = bps.tile([128, GC * H], f32, tag="bps")
                        for off in (0, 64):
                            nc.tensor.matmul(
                                bp[off:off + 64, :],
                                lhsT=onesm[off:off + H, :],
                                rhs=et[off:off + H, :],
                                start=True, stop=True,
                                tile_position=(off, off))
                        # 1/x as exp(-ln(x)) on the idle ScalarE -- DVE
                        # reciprocal is an iterative 8-cyc/elem op; sums
                        # are positive, O(1..1e3): table accuracy is fine
                        lt = rpool.tile([128, GC * H], f32, tag="ob",
                                        bufs=2)
                        nc.scalar.activation(
                            out=lt, in_=bp, func=AF.Ln)
                        rt = rpool.tile([128, GC * H], f32, tag="rt")
                        nc.scalar.activation(
                            out=rt, in_=lt, func=AF.Exp, scale=-1.0)
                        fp = fps.tile([128, GC * H], f32, tag="fps")
                        for ci in range(GC):
                            for off in (0, 64):
                                nc.tensor.matmul(
                                    fp[off:off + W,
                                       ci * H:(ci + 1) * H],
                                    lhsT=VHh[off:off + H, :, c0 + ci],
                                    rhs=et[off:off + H,
                                           ci * H:(ci + 1) * H],
                                    start=True, stop=True,
                                    tile_position=(off, off))
                        # iterate h-outer/c-inner so the FT write is
                        # runs of GC contiguous bf16 (reads strided
                        # instead -- cheaper than strided writes)
                        for off in (0, 64):
                            nc.vector.tensor_tensor(
                                out=FT[:, :, off + c0:off + c0 + GC],
                                in0=fp[off:off + 64].rearrange(
                                    "p (c h) -> p h c", c=GC),
                                in1=rt[off:off + 64].rearrange(
                                    "p (c h) -> p h c", c=GC),
                                op=ALU.mult)
                    if do_xbar and do_attn:
                        fn = fnpool.tile([128, H, WF], bf16, tag="fn")
                        nc.sync.dma_start(
                            out=fn,
                            in_=FT[0:WF, :, :].rearrange("p a b -> p (a b)"),
                            transpose=True)
                        nc.sync.dma_start(out=fnat_d[img, o],
                                          in_=fn[:, :, 0:W])

            # ---- phase C: s-add + conv + borders ----
            # per-kb dependency split: conv matmuls for channel block kb
            # gate only on fuse(kb), so most of the conv runs while the
            # last attention blocks are still finishing
            xmt_i = x_mt[img].rearrange("(kb p) s -> p kb s", p=128)
            fnat_v = fnat_d[img].rearrange("kb p h w -> p kb (h w)")
            for n in (range(NT) if "C" in cfg.phases else []):
                s0b = natpool.tile([128, CB, NS], bf16, tag="nat")
                nc.gpsimd.dma_start(
                    out=s0b, in_=xs_i[:, :, n * NS:(n + 1) * NS])
                xmt_t = xpool.tile([128, CB, NS], bf16, tag="xfq", bufs=2)
                nc.gpsimd.dma_start(
                    out=xmt_t, in_=xmt_i[:, :, n * NS:(n + 1) * NS])
                nc.vector.tensor_tensor(
                    out=s0b, in0=s0b, in1=xmt_t, op=ALU.add)
                fr = xpool.tile([128, CB, NS], bf16, tag="xs")
                for kb in range(CB):
                    nc.sync.dma_start(
                        out=fr[:, kb, :],
                        in_=fnat_v[:, kb, n * NS:(n + 1) * NS])
                    nc.vector.tensor_tensor(
                        out=s0b[:, kb, :], in0=s0b[:, kb, :],
                        in1=fr[:, kb, :], op=ALU.add)
                for og in (0, 3):
                    pss = [mps.tile([128, NS], f32, tag="mixps",
                                    name=f"cps_{img}_{n}_{og + j}")
                           for j in range(3)]
                    for kb in range(CB):
                        for j in range(3):
                            o = og + j
                            nc.tensor.matmul(
                                pss[j],
                                lhsT=w1[:, kb, o * 128:(o + 1) * 128],
                                rhs=s0b[:, kb, :],
                                start=(kb == 0), stop=(kb == CB - 1))
                    for j in range(3):
                        o = og + j
                        # full-width rows: border cols 0 / 57 hold b1[o]
                        # so one contiguous DMA per (o, n) writes it all
                        ob = rpool.tile([128, NS], f32, tag="ob", bufs=2)
                        nc.scalar.activation(
                            out=ob, in_=pss[j], func=AF.Identity,
                            bias=b1t[:, o:o + 1])
                        ot = rpool.tile([128, RT, WP], f32, tag="rt")
                        nc.vector.tensor_copy(
                            out=ot[:, :, 0:1],
                            in_=bord[:, o, 0:RT].rearrange(
                                "p (h u) -> p h u", u=1))
                        nc.vector.tensor_copy(
                            out=ot[:, :, WP - 1:WP],
                            in_=bord[:, o, 0:RT].rearrange(
                                "p (h u) -> p h u", u=1))
                        nc.vector.tensor_copy(
                            out=ot[:, :, 1:1 + W],
                            in_=ob.rearrange("p (h w) -> p h w", h=RT))
                        nc.sync.dma_start(
                            out=y[img, o * 128:(o + 1) * 128,
                                  1 + n * RT:1 + (n + 1) * RT, :],
                            in_=ot)
            for o in range(CB):
                yo = y[img, o * 128:(o + 1) * 128]
                # gpsimd: SWDGE casts the bf16 bias rows to f32 on the way
                nc.gpsimd.dma_start(out=yo[:, 0, :], in_=bord[:, o, 0:WP])
                nc.gpsimd.dma_start(
                    out=yo[:, HP - 1, :], in_=bord[:, o, 0:WP])

    # TRN2 allows at most 1 sync wait per instruction (2 on event-semaphore
    # insts). Tile's sem assignment can emit more; run the bacc passes that
    # move matmul waits onto ldweights and split the rest via event sems.
    import bass_rust as _bass_rust
    _bass_rust.move_matmul_waits_to_ldweights(nc.m)
    _bass_rust.generate_event_semaphores(nc)
    return nc


_PROG_CACHE = {}


def get_program():
    if "full" not in _PROG_CACHE:
        _PROG_CACHE["full"] = build_program(Cfg())
    return _PROG_CACHE["full"]


def _prep_in_maps(x_s, x_fq, x_mt, Wq, Wk, Wv, W1, b1):
    x_s = np.asarray(x_s, dtype=np.float32)
    x_fq = np.asarray(x_fq, dtype=np.float32)
    x_mt = np.asarray(x_mt, dtype=np.float32)
    wqT = np.ascontiguousarray(np.asarray(Wq, np.float32).T)
    wkT = np.ascontiguousarray(np.asarray(Wk, np.float32).T)
    wvT = np.ascontiguousarray(np.asarray(Wv, np.float32).T)
    w1T = np.ascontiguousarray(np.asarray(W1, np.float32).T)
    b1 = np.asarray(b1, dtype=np.float32)

    B, C, H, W = x_s.shape
    per = B // N_CORES
    in_maps = []
    for i in range(N_CORES):
        sl = slice(i * per, (i + 1) * per)
        in_maps.append({
            "x_s": np.ascontiguousarray(x_s[sl].reshape(per, C, H * W)),
            "x_fq": np.ascontiguousarray(x_fq[sl].reshape(per, C, H * W)),
            "x_mt": np.ascontiguousarray(x_mt[sl].reshape(per, C, H * W)),
            "wqT": wqT, "wkT": wkT, "wvT": wvT, "w1T": w1T, "b1": b1,
        })
    return in_maps, per, C, H, W


def kernel(x_s, x_fq, x_mt, Wq, Wk, Wv, W1, b1, trace=False):
    from concourse.bass_utils import run_bass_kernel_spmd

    in_maps, per, C, H, W = _prep_in_maps(
        x_s, x_fq, x_mt, Wq, Wk, Wv, W1, b1)
    nc = get_program()
    r = run_bass_kernel_spmd(nc, in_maps, list(range(N_CORES)), trace=trace)
    out = np.concatenate(
        [r.results[i]["y"].reshape(per, C, H + 2, W + 2)
         for i in range(N_CORES)], axis=0).astype(np.float32)
    if trace:
        return out, r
    return out



# revision 35
# speedup vs baseline: 1.0433x; 1.0433x over previous
"""Trainium2 Bass kernel for nn_CMF: per-channel spatial row-attention + 1x1 convs.

Reference (B=16, C=768, H=W=56):
  q = Wq @ x_s ; k = Wk @ x_fq ; v = Wv @ x_fq        (1x1 convs)
  scores[b,c,h,g] = sum_w q[b,c,h,w] k[b,c,g,w] * (H*W*C)**-0.5
  attn = softmax(scores, -1); fuse = attn @ v
  out = W1 @ zero_pad(x_s + x_mt + fuse, 1) + b1      -> (B, C, 58, 58)

Sharding: data-parallel over batch; 2 images per core on 8 cores (SPMD).

Per-core pipeline (per image, channel blocks of 128 processed in halves):
  A) channel-mix matmuls (bf16) in natural layout -> q/k/v nat tiles
  B) pad-copy (GpSimd) to w128-padded staging; xbar DMA-transpose to
     QT/KT [w(part), h, c] and VH [h(part), w, c]; per-channel attention:
       scoresT = kT.T @ qT  ->  exp(scale*x) on ACT  ->
       sums broadcast to all partitions via all-ones matmul -> reciprocal ->
       fuseT = v.T-form matmul; normalize+scatter into FT [w, h, c]
     back-xbar to natural [c, h, w64]; bounce via DRAM
  C) s = x_s + x_mt + fuse; conv (bf16) + bias on ACT; border rows/cols = b1
"""

import sys

import numpy as np

sys.path.insert(0, "/opt/trn_rl_repo")

N_CORES = 8


class Cfg:
    def __init__(self, imgs=2, cb=6, h=56, w=56, rows_per_tile=8, half=2,
                 nat_bufs=6, sim_safe=False, phases="ABC"):
        self.sim_safe = sim_safe
        self.phases = phases
        self.imgs = imgs
        self.cb = cb
        self.C = cb * 128
        self.H = h
        self.W = w
        self.S = h * w
        self.RT = rows_per_tile
        assert h % rows_per_tile == 0
        self.NT = h // rows_per_tile
        self.NS = rows_per_tile * w
        assert self.NS <= 512
        self.GC = max(1, min(512 // h, 8))
        assert 128 % self.GC == 0
        self.NG = 128 // self.GC
        self.scale = float((h * w * self.C) ** -0.5)
        self.HP = h + 2
        self.WP = w + 2
        self.half = half
        self.nat_bufs = nat_bufs


def build_program(cfg):
    from contextlib import ExitStack

    import concourse.bass as bass
    import concourse.mybir as mybir
    import concourse.tile as tile

    f32 = mybir.dt.float32
    bf16 = mybir.dt.bfloat16
    AF = mybir.ActivationFunctionType
    ALU = mybir.AluOpType

    nc = bass.Bass()

    IM, CB, H, W, S = cfg.imgs, cfg.cb, cfg.H, cfg.W, cfg.S
    RT, NT, NS, C, HP = cfg.RT, cfg.NT, cfg.NS, cfg.C, cfg.HP
    WP = cfg.WP
    GC, NG = cfg.GC, cfg.NG
    WF = 64 if W <= 64 else 128  # fuse natural w stride (back-xbar minor dim)

    x_s = nc.declare_dram_parameter("x_s", [IM, C, S], f32, isOutput=False)
    x_fq = nc.declare_dram_parameter("x_fq", [IM, C, S], f32, isOutput=False)
    x_mt = nc.declare_dram_parameter("x_mt", [IM, C, S], f32, isOutput=False)
    wqT = nc.declare_dram_parameter("wqT", [C, C], f32, isOutput=False)
    wkT = nc.declare_dram_parameter("wkT", [C, C], f32, isOutput=False)
    wvT = nc.declare_dram_parameter("wvT", [C, C], f32, isOutput=False)
    w1T = nc.declare_dram_parameter("w1T", [C, C], f32, isOutput=False)
    b1 = nc.declare_dram_parameter("b1", [C], f32, isOutput=False)
    y = nc.declare_dram_parameter("y", [IM, C, HP, WP], f32, isOutput=True)

    fnat_d = nc.dram_tensor("fnat_d", [IM, CB, 128, H, W], bf16)

    with tile.TileContext(nc) as tc, ExitStack() as ex:
        wpool = ex.enter_context(tc.tile_pool(name="wpool", bufs=1))
        xpool = ex.enter_context(tc.tile_pool(name="xpool", bufs=2))
        natpool = ex.enter_context(tc.tile_pool(name="natpool", bufs=2))
        sgpool = ex.enter_context(tc.tile_pool(name="sgpool", bufs=4))
        bigpool = ex.enter_context(tc.tile_pool(name="bigpool", bufs=2))
        tpool = ex.enter_context(tc.tile_pool(name="tpool", bufs=1))
        epool = ex.enter_context(tc.tile_pool(name="epool", bufs=2))
        rpool = ex.enter_context(tc.tile_pool(name="rpool", bufs=2))
        ftpool = ex.enter_context(tc.tile_pool(name="ftpool", bufs=1))
        fnpool = ex.enter_context(tc.tile_pool(name="fnpool", bufs=1))
        mps = ex.enter_context(tc.tile_pool(name="mps", bufs=3, space="PSUM"))
        sps = ex.enter_context(tc.tile_pool(name="sps", bufs=2, space="PSUM"))
        fps = ex.enter_context(tc.tile_pool(name="fps", bufs=2, space="PSUM"))
        bps = ex.enter_context(tc.tile_pool(name="bps", bufs=1, space="PSUM"))

        # ---- resident weights / constants ----
        wq = wpool.tile([128, CB, C], bf16)
        wk = wpool.tile([128, CB, C], bf16)
        wv = wpool.tile([128, CB, C], bf16)
        w1 = wpool.tile([128, CB, C], bf16)
        for dst, src in ((wq, wqT), (wk, wkT), (wv, wvT), (w1, w1T)):
            nc.gpsimd.dma_start(
                out=dst, in_=src.rearrange("(kb p) o -> p kb o", p=128))
        b1t = wpool.tile([128, CB], f32)
        nc.gpsimd.dma_start(out=b1t, in_=b1.rearrange("(kb p) -> p kb", p=128))
        onesm = wpool.tile([128, 64], bf16)
        nc.vector.memset(onesm, 1.0)
        BW = max(2 * H, WP)
        bord = wpool.tile([128, CB, BW], f32)
        nc.vector.tensor_copy(
            out=bord,
            in_=bass.AP(tensor=b1t.tensor, offset=b1t.offset,
                        ap=[list(b1t.ap[0]), list(b1t.ap[1]), [0, BW]]))

        for img in range(IM):
            xs_i = x_s[img].rearrange("(kb p) s -> p kb s", p=128)
            xfq_i = x_fq[img].rearrange("(kb p) s -> p kb s", p=128)
            for h0 in range(0, CB, cfg.half):
                obs = list(range(h0, min(h0 + cfg.half, CB)))
                nats = {}
                stgs = {}
                # ---- phase A: channel mix ----
                # q/k: stage one n-tile in a small rotating buffer and
                # xbar it immediately into QT/KT -- the transposes
                # pipeline behind the mix matmuls instead of serializing
                # as one big per-o transpose after phase A
                for n in range(NT):
                    xs_t = xpool.tile([128, CB, NS], bf16, tag="xs")
                    xfq_t = xpool.tile([128, CB, NS], bf16, tag="xfq", bufs=2)
                    nc.gpsimd.dma_start(
                        out=xs_t, in_=xs_i[:, :, n * NS:(n + 1) * NS])
                    nc.gpsimd.dma_start(
                        out=xfq_t, in_=xfq_i[:, :, n * NS:(n + 1) * NS])
                    for o in obs:
                        if n == 0:
                            nats[o] = (
                                tpool.tile([128, H, 64], bf16, tag="qt",
                                           bufs=2, name=f"QT_{img}_{o}"),
                                tpool.tile([128, H, 64], bf16, tag="kt",
                                           bufs=2, name=f"KT_{img}_{o}"),
                                natpool.tile([64, S], bf16, tag="nat",
                                             name=f"vnat_{img}_{o}"),
                                natpool.tile([64, S], bf16, tag="nat2",
                                             bufs=2, name=f"vn2_{img}_{o}"),
                                bigpool.tile([64, W, 128], bf16, tag="big",
                                             name=f"vstg_{img}_{o}"),
                            )
                        QT, KT, vn, vn2, vstg = nats[o]
                        for ti, (wmat, xt) in enumerate(
                                ((wq, xs_t), (wk, xfq_t), (wv, xfq_t))):
                            ps = mps.tile([128, NS], f32, tag="mixps")
                            for kb in range(CB):
                                nc.tensor.matmul(
                                    ps,
                                    lhsT=wmat[:, kb, o * 128:(o + 1) * 128],
                                    rhs=xt[:, kb, :],
                                    start=(kb == 0), stop=(kb == CB - 1))
                            if ti == 2:
                                nc.vector.tensor_copy(
                                    out=vn[:, n * NS:(n + 1) * NS],
                                    in_=ps[0:64])
                                nc.vector.tensor_copy(
                                    out=vn2[:, n * NS:(n + 1) * NS],
                                    in_=ps[64:128])
                                # scatter v rows into the h-packed
                                # transpose staging as they arrive
                                nc.gpsimd.tensor_copy(
                                    out=vstg[:, :, n * RT:(n + 1) * RT],
                                    in_=vn[:, n * NS:(n + 1) * NS
                                           ].rearrange(
                                        "p (h w) -> p w h", h=RT))
                                nc.gpsimd.tensor_copy(
                                    out=vstg[:, :,
                                             64 + n * RT:64 + (n + 1) * RT],
                                    in_=vn2[:, n * NS:(n + 1) * NS
                                            ].rearrange(
                                        "p (h w) -> p w h", h=RT))
                            else:
                                # channel-pair packing: channels 0-63 at
                                # w-cols 0:56, 64-127 at 64:64+56, so one
                                # xbar slice covers all 128 channels.
                                # Stage TWO n-tiles per xbar: halves the
                                # sync-engine trigger count (~2.2us each)
                                # while keeping the phase-A overlap
                                if n % 2 == 0:
                                    stgs[(o, ti)] = sgpool.tile(
                                        [64, 2 * RT, 128], bf16, tag="sg",
                                        name=f"stg_{img}_{o}_{ti}_{n}")
                                stg = stgs[(o, ti)]
                                hoff = (n % 2) * RT
                                nc.vector.tensor_copy(
                                    out=stg[:, hoff:hoff + RT, 0:W],
                                    in_=ps[0:64].rearrange(
                                        "p (h w) -> p h w", h=RT))
                                nc.vector.tensor_copy(
                                    out=stg[:, hoff:hoff + RT, 64:64 + W],
                                    in_=ps[64:128].rearrange(
                                        "p (h w) -> p h w", h=RT))
                                if n % 2 == 1 or n == NT - 1:
                                    n0 = (n // 2) * 2
                                    rows = (n - n0 + 1) * RT
                                    nc.sync.dma_start(
                                        out=nats[o][ti][
                                            :, n0 * RT:n0 * RT + rows, :],
                                        in_=stg[:, 0:rows, :].rearrange(
                                            "p a b -> p (a b)"),
                                        transpose=True)
                # ---- phase B: attention ----
                # channel halves A (0-63) / B (64-127) sit at partition
                # offsets 0 / 64 of QT/KT/VHh; A and B matmuls alternate
                # row+col groups so LDWEIGHTS overlaps in-flight matmuls
                for o in (obs if "B" in cfg.phases else []):
                    QT, KT, vn, vn2, vstg = nats[o]
                    do_xbar = "x" not in cfg.phases
                    do_attn = "a" not in cfg.phases
                    FT = ftpool.tile([64, H, 128], bf16, tag="ft")
                    VHh = tpool.tile([128, W, 64], bf16, tag="vh", bufs=2)
                    if do_xbar:
                        nc.sync.dma_start(
                            out=VHh,
                            in_=vstg.rearrange("p a b -> p (a b)"),
                            transpose=True)
                    for g in (range(NG // 2) if (do_xbar and do_attn)
                              else []):
                        c0 = g * GC
                        sp = sps.tile([128, GC * H], f32, tag="sps")
                        for ci in range(GC):
                            for off in (0, 64):
                                nc.tensor.matmul(
                                    sp[off:off + H,
                                       ci * H:(ci + 1) * H],
                                    lhsT=KT[off:off + W, :, c0 + ci],
                                    rhs=QT[off:off + W, :, c0 + ci],
                                    start=True, stop=True,
                                    tile_position=(off, off))
                        et = epool.tile([128, GC * H], bf16, tag="exp",
                                        bufs=2)
                        nc.scalar.activation(
                            out=et, in_=sp, func=AF.Exp, scale=cfg.scale)
                        bp = bps.tile([128, GC * H], f32, tag="bps")
                        for off in (0, 64):
                            nc.tensor.matmul(
                                bp[off:off + 64, :],
                                lhsT=onesm[off:off + H, :],
                                rhs=et[off:off + H, :],
                                start=True, stop=True,
                                tile_position=(off, off))
                        # 1/x as exp(-ln(x)) on the idle ScalarE -- DVE
                        # reciprocal is an iterative 8-cyc/elem op; sums
                        # are positive, O(1..1e3): table accuracy is fine
                        lt = rpool.tile([128, GC * H], f32, tag="ob",
                                        bufs=2)
                        nc.scalar.activation(
                            out=lt, in_=bp, func=AF.Ln)
                        rt = rpool.tile([128, GC * H], f32, tag="rt")
                        nc.scalar.activation(
                            out=rt, in_=lt, func=AF.Exp, scale=-1.0)
                        fp = fps.tile([128, GC * H], f32, tag="fps")
                        for ci in range(GC):
                            for off in (0, 64):
                                nc.tensor.matmul(
                                    fp[off:off + W,
                                       ci * H:(ci + 1) * H],
                                    lhsT=VHh[off:off + H, :, c0 + ci],
                                    rhs=et[off:off + H,
                                           ci * H:(ci + 1) * H],
                                    start=True, stop=True,
                                    tile_position=(off, off))
                        # iterate h-outer/c-inner so the FT write is
                        # runs of GC contiguous bf16 (reads strided
                        # instead -- cheaper than strided writes)
                        for off in (0, 64):
                            nc.vector.tensor_tensor(
                                out=FT[:, :, off + c0:off + c0 + GC],
                                in0=fp[off:off + 64].rearrange(
                                    "p (c h) -> p h c", c=GC),
                                in1=rt[off:off + 64].rearrange(
                                    "p (c h) -> p h c", c=GC),
                                op=ALU.mult)
                    if do_xbar and do_attn:
                        fn = fnpool.tile([128, H, WF], bf16, tag="fn")
                        nc.sync.dma_start(
                            out=fn,
                            in_=FT[0:WF, :, :].rearrange("p a b -> p (a b)"),
                            transpose=True)
                        nc.sync.dma_start(out=fnat_d[img, o],
                                          in_=fn[:, :, 0:W])

            # ---- phase C: s-add + conv + borders ----
            # per-kb dependency split: conv matmuls for channel block kb
            # gate only on fuse(kb), so most of the conv runs while the
            # last attention blocks are still finishing
            xmt_i = x_mt[img].rearrange("(kb p) s -> p kb s", p=128)
            fnat_v = fnat_d[img].rearrange("kb p h w -> p kb (h w)")
            for n in (range(NT) if "C" in cfg.phases else []):
                s0b = natpool.tile([128, CB, NS], bf16, tag="nat")
                nc.gpsimd.dma_start(
                    out=s0b, in_=xs_i[:, :, n * NS:(n + 1) * NS])
                xmt_t = xpool.tile([128, CB, NS], bf16, tag="xfq", bufs=2)
                nc.gpsimd.dma_start(
                    out=xmt_t, in_=xmt_i[:, :, n * NS:(n + 1) * NS])
                nc.vector.tensor_tensor(
                    out=s0b, in0=s0b, in1=xmt_t, op=ALU.add)
                fr = xpool.tile([128, CB, NS], bf16, tag="xs")
                for kb in range(CB):
                    nc.sync.dma_start(
                        out=fr[:, kb, :],
                        in_=fnat_v[:, kb, n * NS:(n + 1) * NS])
                    nc.vector.tensor_tensor(
                        out=s0b[:, kb, :], in0=s0b[:, kb, :],
                        in1=fr[:, kb, :], op=ALU.add)
                for og in (0, 3):
                    pss = [mps.tile([128, NS], f32, tag="mixps",
                                    name=f"cps_{img}_{n}_{og + j}")
                           for j in range(3)]
                    for kb in range(CB):
                        for j in range(3):
                            o = og + j
                            nc.tensor.matmul(
                                pss[j],
                                lhsT=w1[:, kb, o * 128:(o + 1) * 128],
                                rhs=s0b[:, kb, :],
                                start=(kb == 0), stop=(kb == CB - 1))
                    for j in range(3):
                        o = og + j
                        # full-width rows: border cols 0 / 57 hold b1[o]
                        # so one contiguous DMA per (o, n) writes it all
                        ob = rpool.tile([128, NS], f32, tag="ob", bufs=2)
                        nc.scalar.activation(
                            out=ob, in_=pss[j], func=AF.Identity,
                            bias=b1t[:, o:o + 1])
                        ot = rpool.tile([128, RT, WP], f32, tag="rt")
                        nc.vector.tensor_copy(
                            out=ot[:, :, 0:1],
                            in_=bord[:, o, 0:RT].rearrange(
                                "p (h u) -> p h u", u=1))
                        nc.vector.tensor_copy(
                            out=ot[:, :, WP - 1:WP],
                            in_=bord[:, o, 0:RT].rearrange(
                                "p (h u) -> p h u", u=1))
                        nc.vector.tensor_copy(
                            out=ot[:, :, 1:1 + W],
                            in_=ob.rearrange("p (h w) -> p h w", h=RT))
                        nc.sync.dma_start(
                            out=y[img, o * 128:(o + 1) * 128,
                                  1 + n * RT:1 + (n + 1) * RT, :],
                            in_=ot)
            for o in range(CB):
                yo = y[img, o * 128:(o + 1) * 128]
                nc.sync.dma_start(out=yo[:, 0, :], in_=bord[:, o, 0:WP])
                nc.sync.dma_start(out=yo[:, HP - 1, :], in_=bord[:, o, 0:WP])

    # TRN2 allows at most 1 sync wait per instruction (2 on event-semaphore
    # insts). Tile's sem assignment can emit more; run the bacc passes that
    # move matmul waits onto ldweights and split the rest via event sems.
    import bass_rust as _bass_rust
    _bass_rust.move_matmul_waits_to_ldweights(nc.m)
    _bass_rust.generate_event_semaphores(nc)
    return nc


_PROG_CACHE = {}


def get_program():
    if "full" not in _PROG_CACHE:
        _PROG_CACHE["full"] = build_program(Cfg())
    return _PROG_CACHE["full"]


def _prep_in_maps(x_s, x_fq, x_mt, Wq, Wk, Wv, W1, b1):
    x_s = np.asarray(x_s, dtype=np.float32)
    x_fq = np.asarray(x_fq, dtype=np.float32)
    x_mt = np.asarray(x_mt, dtype=np.float32)
    wqT = np.ascontiguousarray(np.asarray(Wq, np.float32).T)
    wkT = np.ascontiguousarray(np.asarray(Wk, np.float32).T)
    wvT = np.ascontiguousarray(np.asarray(Wv, np.float32).T)
    w1T = np.ascontiguousarray(np.asarray(W1, np.float32).T)
    b1 = np.asarray(b1, dtype=np.float32)

    B, C, H, W = x_s.shape
    per = B // N_CORES
    in_maps = []
    for i in range(N_CORES):
        sl = slice(i * per, (i + 1) * per)
        in_maps.append({
            "x_s": np.ascontiguousarray(x_s[sl].reshape(per, C, H * W)),
            "x_fq": np.ascontiguousarray(x_fq[sl].reshape(per, C, H * W)),
            "x_mt": np.ascontiguousarray(x_mt[sl].reshape(per, C, H * W)),
            "wqT": wqT, "wkT": wkT, "wvT": wvT, "w1T": w1T, "b1": b1,
        })
    return in_maps, per, C, H, W


def kernel(x_s, x_fq, x_mt, Wq, Wk, Wv, W1, b1, trace=False):
    from concourse.bass_utils import run_bass_kernel_spmd

    in_maps, per, C, H, W = _prep_in_maps(
        x_s, x_fq, x_mt, Wq, Wk, Wv, W1, b1)
    nc = get_program()
    r = run_bass_kernel_spmd(nc, in_maps, list(range(N_CORES)), trace=trace)
    out = np.concatenate(
        [r.results[i]["y"].reshape(per, C, H + 2, W + 2)
         for i in range(N_CORES)], axis=0).astype(np.float32)
    if trace:
        return out, r
    return out



# revision 37
# speedup vs baseline: 1.0460x; 1.0026x over previous
"""Trainium2 Bass kernel for nn_CMF: per-channel spatial row-attention + 1x1 convs.

Reference (B=16, C=768, H=W=56):
  q = Wq @ x_s ; k = Wk @ x_fq ; v = Wv @ x_fq        (1x1 convs)
  scores[b,c,h,g] = sum_w q[b,c,h,w] k[b,c,g,w] * (H*W*C)**-0.5
  attn = softmax(scores, -1); fuse = attn @ v
  out = W1 @ zero_pad(x_s + x_mt + fuse, 1) + b1      -> (B, C, 58, 58)

Sharding: data-parallel over batch; 2 images per core on 8 cores (SPMD).

Per-core pipeline (per image, channel blocks of 128 processed in halves):
  A) channel-mix matmuls (bf16) in natural layout -> q/k/v nat tiles
  B) pad-copy (GpSimd) to w128-padded staging; xbar DMA-transpose to
     QT/KT [w(part), h, c] and VH [h(part), w, c]; per-channel attention:
       scoresT = kT.T @ qT  ->  exp(scale*x) on ACT  ->
       sums broadcast to all partitions via all-ones matmul -> reciprocal ->
       fuseT = v.T-form matmul; normalize+scatter into FT [w, h, c]
     back-xbar to natural [c, h, w64]; bounce via DRAM
  C) s = x_s + x_mt + fuse; conv (bf16) + bias on ACT; border rows/cols = b1
"""

import sys

import numpy as np

sys.path.insert(0, "/opt/trn_rl_repo")

N_CORES = 8


class Cfg:
    def __init__(self, imgs=2, cb=6, h=56, w=56, rows_per_tile=8, half=2,
                 nat_bufs=6, sim_safe=False, phases="ABC"):
        self.sim_safe = sim_safe
        self.phases = phases
        self.imgs = imgs
        self.cb = cb
        self.C = cb * 128
        self.H = h
        self.W = w
        self.S = h * w
        self.RT = rows_per_tile
        assert h % rows_per_tile == 0
        self.NT = h // rows_per_tile
        self.NS = rows_per_tile * w
        assert self.NS <= 512
        self.GC = max(1, min(512 // h, 8))
        assert 128 % self.GC == 0
        self.NG = 128 // self.GC
        self.scale = float((h * w * self.C) ** -0.5)
        self.HP = h + 2
        self.WP = w + 2
        self.half = half
        self.nat_bufs = nat_bufs


def build_program(cfg):
    from contextlib import ExitStack

    import concourse.bass as bass
    import concourse.mybir as mybir
    import concourse.tile as tile

    f32 = mybir.dt.float32
    bf16 = mybir.dt.bfloat16
    AF = mybir.ActivationFunctionType
    ALU = mybir.AluOpType

    nc = bass.Bass()

    IM, CB, H, W, S = cfg.imgs, cfg.cb, cfg.H, cfg.W, cfg.S
    RT, NT, NS, C, HP = cfg.RT, cfg.NT, cfg.NS, cfg.C, cfg.HP
    WP = cfg.WP
    GC, NG = cfg.GC, cfg.NG
    WF = 64 if W <= 64 else 128  # fuse natural w stride (back-xbar minor dim)

    x_s = nc.declare_dram_parameter("x_s", [IM, C, S], f32, isOutput=False)
    x_fq = nc.declare_dram_parameter("x_fq", [IM, C, S], f32, isOutput=False)
    x_mt = nc.declare_dram_parameter("x_mt", [IM, C, S], f32, isOutput=False)
    wqT = nc.declare_dram_parameter("wqT", [C, C], f32, isOutput=False)
    wkT = nc.declare_dram_parameter("wkT", [C, C], f32, isOutput=False)
    wvT = nc.declare_dram_parameter("wvT", [C, C], f32, isOutput=False)
    w1T = nc.declare_dram_parameter("w1T", [C, C], f32, isOutput=False)
    b1 = nc.declare_dram_parameter("b1", [C], f32, isOutput=False)
    y = nc.declare_dram_parameter("y", [IM, C, HP, WP], f32, isOutput=True)

    fnat_d = nc.dram_tensor("fnat_d", [IM, CB, 128, H, W], bf16)

    with tile.TileContext(nc) as tc, ExitStack() as ex:
        wpool = ex.enter_context(tc.tile_pool(name="wpool", bufs=1))
        xpool = ex.enter_context(tc.tile_pool(name="xpool", bufs=2))
        natpool = ex.enter_context(tc.tile_pool(name="natpool", bufs=2))
        sgpool = ex.enter_context(tc.tile_pool(name="sgpool", bufs=4))
        bigpool = ex.enter_context(tc.tile_pool(name="bigpool", bufs=2))
        tpool = ex.enter_context(tc.tile_pool(name="tpool", bufs=1))
        epool = ex.enter_context(tc.tile_pool(name="epool", bufs=2))
        rpool = ex.enter_context(tc.tile_pool(name="rpool", bufs=2))
        ftpool = ex.enter_context(tc.tile_pool(name="ftpool", bufs=1))
        fnpool = ex.enter_context(tc.tile_pool(name="fnpool", bufs=1))
        mps = ex.enter_context(tc.tile_pool(name="mps", bufs=3, space="PSUM"))
        sps = ex.enter_context(tc.tile_pool(name="sps", bufs=2, space="PSUM"))
        fps = ex.enter_context(tc.tile_pool(name="fps", bufs=2, space="PSUM"))
        bps = ex.enter_context(tc.tile_pool(name="bps", bufs=1, space="PSUM"))

        # ---- resident weights / constants ----
        wq = wpool.tile([128, CB, C], bf16)
        wk = wpool.tile([128, CB, C], bf16)
        wv = wpool.tile([128, CB, C], bf16)
        w1 = wpool.tile([128, CB, C], bf16)
        for dst, src in ((wq, wqT), (wk, wkT), (wv, wvT), (w1, w1T)):
            nc.gpsimd.dma_start(
                out=dst, in_=src.rearrange("(kb p) o -> p kb o", p=128))
        b1t = wpool.tile([128, CB], f32)
        nc.gpsimd.dma_start(out=b1t, in_=b1.rearrange("(kb p) -> p kb", p=128))
        onesm = wpool.tile([128, 64], bf16)
        nc.vector.memset(onesm, 1.0)
        BW = max(2 * H, WP)
        bord = wpool.tile([128, CB, BW], f32)
        nc.vector.tensor_copy(
            out=bord,
            in_=bass.AP(tensor=b1t.tensor, offset=b1t.offset,
                        ap=[list(b1t.ap[0]), list(b1t.ap[1]), [0, BW]]))

        for img in range(IM):
            xs_i = x_s[img].rearrange("(kb p) s -> p kb s", p=128)
            xfq_i = x_fq[img].rearrange("(kb p) s -> p kb s", p=128)
            for h0 in range(0, CB, cfg.half):
                obs = list(range(h0, min(h0 + cfg.half, CB)))
                nats = {}
                stgs = {}
                # ---- phase A: channel mix ----
                # q/k: stage one n-tile in a small rotating buffer and
                # xbar it immediately into QT/KT -- the transposes
                # pipeline behind the mix matmuls instead of serializing
                # as one big per-o transpose after phase A
                for n in range(NT):
                    xs_t = xpool.tile([128, CB, NS], bf16, tag="xs")
                    xfq_t = xpool.tile([128, CB, NS], bf16, tag="xfq", bufs=2)
                    nc.gpsimd.dma_start(
                        out=xs_t, in_=xs_i[:, :, n * NS:(n + 1) * NS])
                    nc.gpsimd.dma_start(
                        out=xfq_t, in_=xfq_i[:, :, n * NS:(n + 1) * NS])
                    for o in obs:
                        if n == 0:
                            nats[o] = (
                                tpool.tile([128, H, 64], bf16, tag="qt",
                                           bufs=2, name=f"QT_{img}_{o}"),
                                tpool.tile([128, H, 64], bf16, tag="kt",
                                           bufs=2, name=f"KT_{img}_{o}"),
                                natpool.tile([64, S], bf16, tag="nat",
                                             name=f"vnat_{img}_{o}"),
                                natpool.tile([64, S], bf16, tag="nat2",
                                             bufs=2, name=f"vn2_{img}_{o}"),
                                bigpool.tile([64, W, 128], bf16, tag="big",
                                             name=f"vstg_{img}_{o}"),
                            )
                        QT, KT, vn, vn2, vstg = nats[o]
                        for ti, (wmat, xt) in enumerate(
                                ((wq, xs_t), (wk, xfq_t), (wv, xfq_t))):
                            ps = mps.tile([128, NS], f32, tag="mixps")
                            for kb in range(CB):
                                nc.tensor.matmul(
                                    ps,
                                    lhsT=wmat[:, kb, o * 128:(o + 1) * 128],
                                    rhs=xt[:, kb, :],
                                    start=(kb == 0), stop=(kb == CB - 1))
                            if ti == 2:
                                nc.vector.tensor_copy(
                                    out=vn[:, n * NS:(n + 1) * NS],
                                    in_=ps[0:64])
                                nc.vector.tensor_copy(
                                    out=vn2[:, n * NS:(n + 1) * NS],
                                    in_=ps[64:128])
                                # scatter v rows into the h-packed
                                # transpose staging, two n-tiles per op
                                # to halve GpSimd per-op overhead (the
                                # same engine drives the x-load queue)
                                if n % 2 == 1 or n == NT - 1:
                                    n0 = (n // 2) * 2
                                    rt2 = (n - n0 + 1) * RT
                                    nc.gpsimd.tensor_copy(
                                        out=vstg[:, :,
                                                 n0 * RT:n0 * RT + rt2],
                                        in_=vn[:, n0 * NS:
                                               n0 * NS + rt2 * W
                                               ].rearrange(
                                            "p (h w) -> p w h", h=rt2))
                                    nc.gpsimd.tensor_copy(
                                        out=vstg[:, :, 64 + n0 * RT:
                                                 64 + n0 * RT + rt2],
                                        in_=vn2[:, n0 * NS:
                                                n0 * NS + rt2 * W
                                                ].rearrange(
                                            "p (h w) -> p w h", h=rt2))
                            else:
                                # channel-pair packing: channels 0-63 at
                                # w-cols 0:56, 64-127 at 64:64+56, so one
                                # xbar slice covers all 128 channels.
                                # Stage TWO n-tiles per xbar: halves the
                                # sync-engine trigger count (~2.2us each)
                                # while keeping the phase-A overlap
                                if n % 2 == 0:
                                    stgs[(o, ti)] = sgpool.tile(
                                        [64, 2 * RT, 128], bf16, tag="sg",
                                        name=f"stg_{img}_{o}_{ti}_{n}")
                                stg = stgs[(o, ti)]
                                hoff = (n % 2) * RT
                                nc.vector.tensor_copy(
                                    out=stg[:, hoff:hoff + RT, 0:W],
                                    in_=ps[0:64].rearrange(
                                        "p (h w) -> p h w", h=RT))
                                nc.vector.tensor_copy(
                                    out=stg[:, hoff:hoff + RT, 64:64 + W],
                                    in_=ps[64:128].rearrange(
                                        "p (h w) -> p h w", h=RT))
                                if n % 2 == 1 or n == NT - 1:
                                    n0 = (n // 2) * 2
                                    rows = (n - n0 + 1) * RT
                                    nc.sync.dma_start(
                                        out=nats[o][ti][
                                            :, n0 * RT:n0 * RT + rows, :],
                                        in_=stg[:, 0:rows, :].rearrange(
                                            "p a b -> p (a b)"),
                                        transpose=True)
                # ---- phase B: attention ----
                # channel halves A (0-63) / B (64-127) sit at partition
                # offsets 0 / 64 of QT/KT/VHh; A and B matmuls alternate
                # row+col groups so LDWEIGHTS overlaps in-flight matmuls
                for o in (obs if "B" in cfg.phases else []):
                    QT, KT, vn, vn2, vstg = nats[o]
                    do_xbar = "x" not in cfg.phases
                    do_attn = "a" not in cfg.phases
                    FT = ftpool.tile([64, H, 128], bf16, tag="ft")
                    VHh = tpool.tile([128, W, 64], bf16, tag="vh", bufs=2)
                    if do_xbar:
                        nc.sync.dma_start(
                            out=VHh,
                            in_=vstg.rearrange("p a b -> p (a b)"),
                            transpose=True)
                    for g in (range(NG // 2) if (do_xbar and do_attn)
                              else []):
                        c0 = g * GC
                        sp = sps.tile([128, GC * H], f32, tag="sps")
                        for ci in range(GC):
                            for off in (0, 64):
                                nc.tensor.matmul(
                                    sp[off:off + H,
                                       ci * H:(ci + 1) * H],
                                    lhsT=KT[off:off + W, :, c0 + ci],
                                    rhs=QT[off:off + W, :, c0 + ci],
                                    start=True, stop=True,
                                    tile_position=(off, off))
                        et = epool.tile([128, GC * H], bf16, tag="exp",
                                        bufs=3)
                        nc.scalar.activation(
                            out=et, in_=sp, func=AF.Exp, scale=cfg.scale)
                        bp = bps.tile([128, GC * H], f32, tag="bps")
                        for off in (0, 64):
                            nc.tensor.matmul(
                                bp[off:off + 64, :],
                                lhsT=onesm[off:off + H, :],
                                rhs=et[off:off + H, :],
                                start=True, stop=True,
                                tile_position=(off, off))
                        # 1/x as exp(-ln(x)) on the idle ScalarE -- DVE
                        # reciprocal is an iterative 8-cyc/elem op; sums
                        # are positive, O(1..1e3): table accuracy is fine
                        lt = rpool.tile([128, GC * H], f32, tag="ob",
                                        bufs=2)
                        nc.scalar.activation(
                            out=lt, in_=bp, func=AF.Ln)
                        rt = rpool.tile([128, GC * H], f32, tag="rt")
                        nc.scalar.activation(
                            out=rt, in_=lt, func=AF.Exp, scale=-1.0)
                        fp = fps.tile([128, GC * H], f32, tag="fps")
                        for ci in range(GC):
                            for off in (0, 64):
                                nc.tensor.matmul(
                                    fp[off:off + W,
                                       ci * H:(ci + 1) * H],
                                    lhsT=VHh[off:off + H, :, c0 + ci],
                                    rhs=et[off:off + H,
                                           ci * H:(ci + 1) * H],
                                    start=True, stop=True,
                                    tile_position=(off, off))
                        # iterate h-outer/c-inner so the FT write is
                        # runs of GC contiguous bf16 (reads strided
                        # instead -- cheaper than strided writes)
                        for off in (0, 64):
                            nc.vector.tensor_tensor(
                                out=FT[:, :, off + c0:off + c0 + GC],
                                in0=fp[off:off + 64].rearrange(
                                    "p (c h) -> p h c", c=GC),
                                in1=rt[off:off + 64].rearrange(
                                    "p (c h) -> p h c", c=GC),
                                op=ALU.mult)
                    if do_xbar and do_attn:
                        fn = fnpool.tile([128, H, WF], bf16, tag="fn")
                        nc.sync.dma_start(
                            out=fn,
                            in_=FT[0:WF, :, :].rearrange("p a b -> p (a b)"),
                            transpose=True)
                        nc.sync.dma_start(out=fnat_d[img, o],
                                          in_=fn[:, :, 0:W])

            # ---- phase C: s-add + conv + borders ----
            # per-kb dependency split: conv matmuls for channel block kb
            # gate only on fuse(kb), so most of the conv runs while the
            # last attention blocks are still finishing
            xmt_i = x_mt[img].rearrange("(kb p) s -> p kb s", p=128)
            fnat_v = fnat_d[img].rearrange("kb p h w -> p kb (h w)")
            for n in (range(NT) if "C" in cfg.phases else []):
                s0b = natpool.tile([128, CB, NS], bf16, tag="nat")
                nc.gpsimd.dma_start(
                    out=s0b, in_=xs_i[:, :, n * NS:(n + 1) * NS])
                xmt_t = xpool.tile([128, CB, NS], bf16, tag="xfq", bufs=2)
                nc.gpsimd.dma_start(
                    out=xmt_t, in_=xmt_i[:, :, n * NS:(n + 1) * NS])
                nc.vector.tensor_tensor(
                    out=s0b, in0=s0b, in1=xmt_t, op=ALU.add)
                fr = xpool.tile([128, CB, NS], bf16, tag="xs")
                for kb in range(CB):
                    nc.sync.dma_start(
                        out=fr[:, kb, :],
                        in_=fnat_v[:, kb, n * NS:(n + 1) * NS])
                    nc.vector.tensor_tensor(
                        out=s0b[:, kb, :], in0=s0b[:, kb, :],
                        in1=fr[:, kb, :], op=ALU.add)
                for og in (0, 3):
                    pss = [mps.tile([128, NS], f32, tag="mixps",
                                    name=f"cps_{img}_{n}_{og + j}")
                           for j in range(3)]
                    for kb in range(CB):
                        for j in range(3):
                            o = og + j
                            nc.tensor.matmul(
                                pss[j],
                                lhsT=w1[:, kb, o * 128:(o + 1) * 128],
                                rhs=s0b[:, kb, :],
                                start=(kb == 0), stop=(kb == CB - 1))
                    for j in range(3):
                        o = og + j
                        # full-width rows: border cols 0 / 57 hold b1[o]
                        # so one contiguous DMA per (o, n) writes it all
                        ob = rpool.tile([128, NS], f32, tag="ob", bufs=2)
                        nc.scalar.activation(
                            out=ob, in_=pss[j], func=AF.Identity,
                            bias=b1t[:, o:o + 1])
                        ot = rpool.tile([128, RT, WP], f32, tag="rt")
                        nc.vector.tensor_copy(
                            out=ot[:, :, 0:1],
                            in_=bord[:, o, 0:RT].rearrange(
                                "p (h u) -> p h u", u=1))
                        nc.vector.tensor_copy(
                            out=ot[:, :, WP - 1:WP],
                            in_=bord[:, o, 0:RT].rearrange(
                                "p (h u) -> p h u", u=1))
                        nc.vector.tensor_copy(
                            out=ot[:, :, 1:1 + W],
                            in_=ob.rearrange("p (h w) -> p h w", h=RT))
                        nc.sync.dma_start(
                            out=y[img, o * 128:(o + 1) * 128,
                                  1 + n * RT:1 + (n + 1) * RT, :],
                            in_=ot)
            for o in range(CB):
                yo = y[img, o * 128:(o + 1) * 128]
                nc.sync.dma_start(out=yo[:, 0, :], in_=bord[:, o, 0:WP])
                nc.sync.dma_start(out=yo[:, HP - 1, :], in_=bord[:, o, 0:WP])

    # TRN2 allows at most 1 sync wait per instruction (2 on event-semaphore
    # insts). Tile's sem assignment can emit more; run the bacc passes that
    # move matmul waits onto ldweights and split the rest via event sems.
    import bass_rust as _bass_rust
    _bass_rust.move_matmul_waits_to_ldweights(nc.m)
    _bass_rust.generate_event_semaphores(nc)
    return nc


_PROG_CACHE = {}


def get_program():
    if "full" not in _PROG_CACHE:
        _PROG_CACHE["full"] = build_program(Cfg())
    return _PROG_CACHE["full"]


def _prep_in_maps(x_s, x_fq, x_mt, Wq, Wk, Wv, W1, b1):
    x_s = np.asarray(x_s, dtype=np.float32)
    x_fq = np.asarray(x_fq, dtype=np.float32)
    x_mt = np.asarray(x_mt, dtype=np.float32)
    wqT = np.ascontiguousarray(np.asarray(Wq, np.float32).T)
    wkT = np.ascontiguousarray(np.asarray(Wk, np.float32).T)
    wvT = np.ascontiguousarray(np.asarray(Wv, np.float32).T)
    w1T = np.ascontiguousarray(np.asarray(W1, np.float32).T)
    b1 = np.asarray(b1, dtype=np.float32)

    B, C, H, W = x_s.shape
    per = B // N_CORES
    in_maps = []
    for i in range(N_CORES):
        sl = slice(i * per, (i + 1) * per)
        in_maps.append({
            "x_s": np.ascontiguousarray(x_s[sl].reshape(per, C, H * W)),
            "x_fq": np.ascontiguousarray(x_fq[sl].reshape(per, C, H * W)),
            "x_mt": np.ascontiguousarray(x_mt[sl].reshape(per, C, H * W)),
            "wqT": wqT, "wkT": wkT, "wvT": wvT, "w1T": w1T, "b1": b1,
        })
    return in_maps, per, C, H, W


def kernel(x_s, x_fq, x_mt, Wq, Wk, Wv, W1, b1, trace=False):
    from concourse.bass_utils import run_bass_kernel_spmd

    in_maps, per, C, H, W = _prep_in_maps(
        x_s, x_fq, x_mt, Wq, Wk, Wv, W1, b1)
    nc = get_program()
    r = run_bass_kernel_spmd(nc, in_maps, list(range(N_CORES)), trace=trace)
    out = np.concatenate(
        [r.results[i]["y"].reshape(per, C, H + 2, W + 2)
         for i in range(N_CORES)], axis=0).astype(np.float32)
    if trace:
        return out, r
    return out

